# revision 32
# baseline (speedup 1.0000x reference)
"""Trainium2 Bass kernel for nn_MultiHeadNetwork (moe_routing).

Strategy
--------
Host side (numpy, inside kernel()):
  * task id per row = argmax of the trailing one-hot block of x (data, not
    activation dependent).  Rows are bin-packed by task into 32 blocks of 128
    rows (~2-3 distinct tasks each); blocks are then assigned to cores sorted
    by task count so a single shared per-position slot profile (e.g.
    (3, 3, 2, 2)) covers every core with minimal padding.
  * Trunk weights replicated across cores.  Per core the head only needs the
    few tasks its blocks span: per (block, slot) the spanned task's head
    weights are packed (the slot weights / masks are per-core data; the
    program structure is the shared profile, so SPMD-uniform).
  * Everything is converted to bfloat16 on host (halves DMA + H2D vs fp32;
    the PE runs bf16 at the same 1 col/cycle rate as fp32r, and bf16 enables
    fast weight loads).
  * The one-hot block of x contributes W0[2048+tid] + b0 per row, so layer 0
    runs over the 2048 dense features only (16 k-chunks instead of 17); the
    per-row contribution is host-gathered and added on the vector engine.

Device side (one SPMD Tile program on 8 cores):
  * Activations feature-major (hT: [feat partitions, batch free]) so each
    trunk layer is out = W_chunk.T @ hT with NO transposes anywhere.
  * Trunk layers run k-OUTER over 8-chunk groups (8 PSUM banks): weight DMAs
    are [128, 1024] bf16 (2 KiB / partition line), prefetched 14 deep; ReLU +
    bias fused on the scalar engine straight out of PSUM (layer 0: DVE add of
    the one-hot contribution + ReLU), output bf16.
  * A few dummy warm-up matmuls run during the first weight DMA so the PE
    HAM clock-gate is at full rate when the real work arrives.
  * Head: per 128-row block b and slot s: psum[row, h] accumulates
    h3[k][:, block cols].T @ head_W[slot][k-chunk] over the 16 k-chunks
    (the h3 block is the *stationary* operand).  copy_predicated with a host
    0/1 row mask merges each slot's rows into the block output.  All head
    weight DMAs are issued early so they complete during the trunk.
  * Head bias and the inverse permutation are applied on host.
"""

import numpy as np
import ml_dtypes
from contextlib import ExitStack

import concourse.bacc as bacc
import concourse.mybir as mybir
from concourse.tile import TileContext
from concourse import bass_utils

BATCH = 4096
FEAT = 2048
NUM_TASKS = 50
WIDTH = 2048
HEAD_DIM = 256
NCORES = 8
BPC = BATCH // NCORES          # 512 rows per core
BLK = 128                      # head row-block size
NBLK = BPC // BLK              # 4 blocks per core
KC = WIDTH // 128              # 16 contraction chunks per layer
WC = WIDTH // 128              # 16 w-chunks per layer
GW = 8                         # w-chunks per trunk group (8 PSUM banks)
NG = WC // GW                  # 2 groups per layer

F32 = mybir.dt.float32
BF16 = mybir.dt.bfloat16
U8 = mybir.dt.uint8
NPBF16 = ml_dtypes.bfloat16

_PROG_CACHE: dict = {}
_PACK_CACHE: dict = {}


def _fingerprint(*arrs):
    """Cheap identity+content fingerprint for caching packed weights."""
    parts = []
    for a in arrs:
        s = np.asarray(a).reshape(-1)
        step = max(1, s.size // 64)
        parts.append((id(a), s.size, float(s[::step][:64].astype(np.float64).sum())))
    return tuple(parts)


def _build(profile, repeat: int = 1, bench: bool = False):
    """Build + compile the SPMD Tile program.

    profile[p] = head slots for the core's p-th 128-row block (blocks are
    assigned to cores so that per-position task counts fit a single shared
    profile, e.g. (3, 3, 2, 2) -- smaller than a uniform max).  An int is
    accepted as a uniform profile.  repeat > 1 wraps the body in a hardware
    For_i loop (benchmarking only).  bench=True turns the big inputs into
    Internal (device-resident scratch) tensors so a benchmark run has no big
    H2D transfers.
    """
    if isinstance(profile, int):
        profile = (profile,) * NBLK
    sbase = [sum(profile[:p]) for p in range(NBLK)]
    nslot = sum(profile)
    nc = bacc.Bacc("TRN2", target_bir_lowering=False, debug=False)
    kind = "Internal" if bench else "ExternalInput"
    xT = nc.dram_tensor("xT", [KC, 128, BPC], BF16, kind=kind).ap()
    ctb = nc.dram_tensor("ctb", [KC, 128, BPC], BF16, kind=kind).ap()
    w0 = nc.dram_tensor("w0p", [NG, KC, 128, GW * 128], BF16, kind=kind).ap()
    w1 = nc.dram_tensor("w1p", [NG, KC, 128, GW * 128], BF16, kind=kind).ap()
    w2 = nc.dram_tensor("w2p", [NG, KC, 128, GW * 128], BF16, kind=kind).ap()
    bia = nc.dram_tensor("bias", [128, 3 * WC], F32, kind=kind).ap()
    hws = nc.dram_tensor("hws", [nslot, 128, KC * HEAD_DIM], BF16, kind=kind).ap()
    msk = nc.dram_tensor("msk", [128, nslot * HEAD_DIM], U8, kind=kind).ap()
    if bench:
        dummy = nc.dram_tensor("bmark_in", [128, 16], F32, kind="ExternalInput").ap()
        outk = "Internal"
    else:
        outk = "ExternalOutput"
    out = nc.dram_tensor("outT", [NBLK, 128, HEAD_DIM], F32, kind=outk).ap()
    if bench:
        outb = nc.dram_tensor("outb", [128, 16], F32, kind="ExternalOutput").ap()

    with TileContext(nc) as tc, ExitStack() as ctx:
        # xT (16 tiles) and h2 (16) share slots: h2 allocates only after
        # layer 0 fully finished reading xT.  h1/h3 share the other pool.
        actA = ctx.enter_context(tc.tile_pool(name="actA", bufs=KC))
        actB = ctx.enter_context(tc.tile_pool(name="actB", bufs=KC))
        wp = ctx.enter_context(tc.tile_pool(name="wp", bufs=14))
        cons = ctx.enter_context(tc.tile_pool(name="cons", bufs=1))
        ctp = ctx.enter_context(tc.tile_pool(name="ctp", bufs=10))
        hwp = ctx.enter_context(tc.tile_pool(name="hwp", bufs=nslot))
        op = ctx.enter_context(tc.tile_pool(name="op", bufs=4))
        psp = ctx.enter_context(tc.tile_pool(name="psp", bufs=8, space="PSUM"))

        if bench:
            # one-time (outside the loop) init of the Internal scratch
            # tensors: uninitialized HBM can hold NaN/denormal bit patterns
            # that would skew the timing vs real data
            with tc.tile_pool(name="initp", bufs=1) as initp:
                zt = initp.tile([128, GW * 128], BF16, tag="zt")
                nc.vector.memset(zt[:], 0.125)
                for g in range(NG):
                    for k in range(KC):
                        nc.sync.dma_start(w0[g, k], zt[:])
                        nc.sync.dma_start(w1[g, k], zt[:])
                        nc.sync.dma_start(w2[g, k], zt[:])
                for k in range(KC):
                    nc.sync.dma_start(xT[k], zt[:, :BPC])
                    nc.sync.dma_start(ctb[k], zt[:, :BPC])
                for sl in range(nslot):
                    for k in range(KC):
                        nc.sync.dma_start(
                            hws[sl][:, k * HEAD_DIM:(k + 1) * HEAD_DIM],
                            zt[:, :HEAD_DIM])
                ztf = initp.tile([128, 3 * WC], F32, tag="ztf")
                nc.vector.memset(ztf[:], 0.0)
                nc.sync.dma_start(bia, ztf[:])
                ztm = initp.tile([128, nslot * HEAD_DIM], U8, tag="ztm")
                nc.vector.memset(ztm[:], 1)
                nc.sync.dma_start(msk, ztm[:])

        if repeat > 1:
            ctx.enter_context(tc.For_i(0, repeat, 1))

        xt = [None] * KC

        # head-weight prefetch queue: dripped one DMA per few trunk
        # k-iterations (a single burst would starve the trunk weight DMAs)
        hw = [hwp.tile([128, KC * HEAD_DIM], BF16, tag="hwp", name=f"hw{sl}")
              for sl in range(nslot)]
        mt = cons.tile([128, nslot * HEAD_DIM], U8, tag="mt")
        bt = cons.tile([128, 3 * WC], F32, tag="bt")
        drip_q = [(mt, msk)] + [(hw[sl], hws[sl]) for sl in range(nslot)]

        # PE warm-up: dummy matmuls on a memset tile run during the initial
        # weight-DMA wait, so the HAM clock-gate un-throttles (needs ~3.4 us
        # of sustained PE activity) before the real matmuls begin.
        wu = wp.tile([128, GW * 128], BF16, tag="wp", name="warm")
        nc.vector.memset(wu[:], 0.0)
        wups = psp.tile([128, BPC], F32, tag="ps", name="warmps")
        for i in range(8):
            nc.tensor.matmul(wups[:], wu[:, :128], wu[:, :BPC],
                             start=(i == 0), stop=(i == 7))

        ctt = [None] * KC

        def trunk_layer(src, wdram, nk, li, pool, tag, load_x=False, drip=0):
            outs = [None] * WC
            it = 0
            for g in range(NG):
                pss = [
                    psp.tile([128, BPC], F32, tag="ps", name=f"psL{li}g{g}w{w}")
                    for w in range(GW)
                ]
                for k in range(nk):
                    wt = wp.tile([128, GW * 128], BF16, tag="wp", name=f"wtL{li}g{g}k{k}")
                    if li == 0 and g == 0 and k == 0:
                        # split the very first weight DMA so the first
                        # stationary [128,128] chunk lands as early as possible
                        nc.sync.dma_start(wt[:, :128], wdram[g, k][:, :128])
                        nc.sync.dma_start(wt[:, 128:], wdram[g, k][:, 128:])
                    else:
                        nc.sync.dma_start(wt[:], wdram[g, k])
                    if load_x and g == 0:
                        t = actA.tile([128, BPC], BF16, tag="actA", name=f"xt{k}")
                        nc.sync.dma_start(t[:], xT[k])
                        src[k] = t
                        if k == 0:
                            # bias: small DMA, must be traced before the
                            # first ReLU that reads it
                            nc.sync.dma_start(bt[:], bia)
                    if load_x and k % 2 == 1:
                        # drip the one-hot contribution tiles for this group:
                        # chunk g*GW + (k//2) is needed by this group's ReLU
                        ci = g * GW + k // 2
                        ctt[ci] = ctp.tile([128, BPC], BF16, tag="ctp",
                                           name=f"ct{ci}")
                        nc.sync.dma_start(ctt[ci][:], ctb[ci])
                    if drip and drip_q and it % drip == drip - 1:
                        tile, src_ap = drip_q.pop(0)
                        nc.sync.dma_start(tile[:], src_ap)
                    it += 1
                    for w in range(GW):
                        nc.tensor.matmul(
                            pss[w][:],
                            wt[:, w * 128:(w + 1) * 128],
                            src[k][:],
                            start=(k == 0),
                            stop=(k == nk - 1),
                        )
                for w in range(GW):
                    wc_i = g * GW + w
                    h = pool.tile([128, BPC], BF16, tag=tag, name=f"h{li}_{wc_i}")
                    if li == 0:
                        # layer 0: the one-hot block of x contributes
                        # W0[2048+tid] + b0 per row (host-gathered): add on
                        # DVE straight out of PSUM, then ReLU in place
                        nc.vector.tensor_tensor(
                            h[:], pss[w][:], ctt[wc_i][:], mybir.AluOpType.add)
                        nc.vector.tensor_relu(h[:], h[:])
                    else:
                        nc.scalar.activation(
                            h[:], pss[w][:], mybir.ActivationFunctionType.Relu,
                            bias=bt[:, li * WC + wc_i: li * WC + wc_i + 1],
                        )
                    outs[wc_i] = h
            return outs

        h1 = trunk_layer(xt, w0, KC, 0, actB, "actB", load_x=True)
        h2 = trunk_layer(h1, w1, KC, 1, actA, "actA", drip=2)
        h3 = trunk_layer(h2, w2, KC, 2, actB, "actB", drip=2)
        for tile, src_ap in drip_q:
            nc.sync.dma_start(tile[:], src_ap)
        drip_q.clear()

        for b in range(NBLK):
            ob = op.tile([128, HEAD_DIM], F32, tag="op", name=f"ob{b}")
            for s in range(profile[b]):
                sl = sbase[b] + s
                ps = psp.tile([128, HEAD_DIM], F32, tag="ps", name=f"psH{sl}")
                for k in range(KC):
                    nc.tensor.matmul(
                        ps[:],
                        h3[k][:, b * BLK:(b + 1) * BLK],
                        hw[sl][:, k * HEAD_DIM:(k + 1) * HEAD_DIM],
                        start=(k == 0),
                        stop=(k == KC - 1),
                    )
                if s == 0:
                    nc.vector.tensor_copy(ob[:], ps[:])
                else:
                    nc.vector.copy_predicated(
                        ob[:], mt[:, sl * HEAD_DIM:(sl + 1) * HEAD_DIM], ps[:]
                    )
            nc.sync.dma_start(out[b], ob[:])

        if bench:
            dt = cons.tile([128, 16], F32, tag="dt")
            nc.sync.dma_start(dt[:], dummy)
            nc.vector.tensor_copy(dt[:], ob[:, :16])
            nc.sync.dma_start(outb, dt[:])

    nc.compile()
    return nc


def _pack_w(W, nk):
    # [NG, nk, 128, GW*128]; [g, k, kp, w*128+m] = W[k*128+kp, (g*GW+w)*128+m]
    return np.ascontiguousarray(
        W.reshape(nk, 128, NG, GW * 128).transpose(2, 0, 1, 3)
    )


def _pack_trunk(W0, W1, W2, b1, b2):
    w0p = _pack_w(W0[:FEAT].astype(NPBF16), KC)
    w1p = _pack_w(W1.astype(NPBF16), KC)
    w2p = _pack_w(W2.astype(NPBF16), KC)
    bias = np.zeros((128, 3 * WC), np.float32)
    for li, b in ((1, b1), (2, b2)):
        bias[:, li * WC:(li + 1) * WC] = b.reshape(WC, 128).T
    return w0p, w1p, w2p, bias


def _pack_rows(tid):
    """Bin-pack rows by task into 128-row blocks, then assign blocks to
    cores so that per-position task counts are balanced: the shared slot
    profile becomes e.g. (3, 3, 2, 2) instead of a uniform max of 3.

    Returns (order, core_blocks, profile): order[i] = original row index at
    packed position i (rows laid out core-major, position-major);
    core_blocks[c][p] = task list of core c's p-th block; profile[p] = slot
    count for position p.
    """
    nblk_total = BATCH // BLK
    counts = np.bincount(tid, minlength=NUM_TASKS)
    rem = {t: int(c) for t, c in enumerate(counts) if c > 0}
    # queues of original row indices per task
    row_q = {t: list(np.nonzero(tid == t)[0]) for t in rem}
    blocks = []  # (row_indices, task_list)
    carry = None  # (task, count) remnant that must start the next block
    for b in range(nblk_total):
        cap = BLK
        rows_here = []
        tasks_here = []
        while cap > 0:
            if carry is not None:
                t, c = carry
                carry = None
            elif rem:
                # largest task that fits entirely, else split the largest
                fit = [(c, t) for t, c in rem.items() if c <= cap]
                if fit:
                    c, t = max(fit)
                else:
                    c, t = max((c, t) for t, c in rem.items())
                del rem[t]
            else:
                break
            take = min(c, cap)
            q = row_q[t]
            rows_here.extend(q[:take])
            row_q[t] = q[take:]
            if t not in tasks_here:
                tasks_here.append(t)
            cap -= take
            if c > take:
                carry = (t, c - take)
        blocks.append((rows_here, tasks_here))
    assert carry is None and not rem
    # sort blocks by task count desc; position p of core c gets global rank
    # 8p + c, so within each core counts are non-increasing and profile[p]
    # (the per-position max) is as small as the distribution allows
    ranked = sorted(range(nblk_total), key=lambda b: -len(blocks[b][1]))
    core_blocks = [[None] * NBLK for _ in range(NCORES)]
    order = []
    profile = []
    for p in range(NBLK):
        profile.append(len(blocks[ranked[8 * p]][1]))
    for c in range(NCORES):
        for p in range(NBLK):
            rows, tasks = blocks[ranked[8 * p + c]]
            core_blocks[c][p] = tasks
            order.extend(rows)
    return np.asarray(order), core_blocks, tuple(profile)


def prepare(x, W0, b0, W1, b1, W2, b2, head_W, head_b):
    """Host-side sharding. Returns (in_maps, order, sorted_task_ids, profile)."""
    x = np.asarray(x, np.float32)
    tid = np.argmax(x[:, -NUM_TASKS:], axis=1)
    order, core_blocks, profile = _pack_rows(tid)
    x_s = x[order]
    t_s = tid[order]
    sbase = [sum(profile[:p]) for p in range(NBLK)]

    fp = _fingerprint(W0, W1, W2, b0, b1, b2, head_W)
    cached = _PACK_CACHE.get("w")
    if cached is not None and cached[0] == fp:
        w0p, w1p, w2p, bias, W0oh, hw_pack = cached[1]
    else:
        W0 = np.asarray(W0, np.float32)
        w0p, w1p, w2p, bias = _pack_trunk(
            W0, np.asarray(W1, np.float32), np.asarray(W2, np.float32),
            np.asarray(b1, np.float32), np.asarray(b2, np.float32))
        # one-hot contribution rows: relu(x @ W0 + b0) = relu(feats @
        # W0[:2048] + W0[2048 + tid] + b0) -- last two terms host-gathered
        W0oh = W0[FEAT:FEAT + NUM_TASKS] + np.asarray(b0, np.float32)[None, :]
        head_W = np.asarray(head_W, np.float32).astype(NPBF16)
        # hw_pack[t, kp, kc*256 + j] = head_W[t, kc*128 + kp, j]
        hw_pack = np.ascontiguousarray(
            head_W.reshape(NUM_TASKS, KC, 128, HEAD_DIM)
            .transpose(0, 2, 1, 3)
            .reshape(NUM_TASKS, 128, KC * HEAD_DIM)
        )
        _PACK_CACHE["w"] = (fp, (w0p, w1p, w2p, bias, W0oh, hw_pack))

    nslot = sum(profile)
    in_maps = []
    for c in range(NCORES):
        xs = x_s[c * BPC:(c + 1) * BPC]
        xTp = np.ascontiguousarray(xs[:, :FEAT].T).astype(NPBF16)
        ts_c = t_s[c * BPC:(c + 1) * BPC]
        ct_c = np.ascontiguousarray(W0oh[ts_c].T).astype(NPBF16)
        slot_tasks = []
        msk_c = np.zeros((128, nslot * HEAD_DIM), np.uint8)
        for b in range(NBLK):
            tl = core_blocks[c][b]
            tl_p = tl + [tl[-1]] * (profile[b] - len(tl))
            ch = t_s[c * BPC + b * BLK: c * BPC + (b + 1) * BLK]
            for s, t in enumerate(tl_p):
                sl = sbase[b] + s
                slot_tasks.append(t)
                if 0 < s < len(tl):
                    msk_c[:, sl * HEAD_DIM:(sl + 1) * HEAD_DIM] = \
                        (ch == t)[:, None].astype(np.uint8)
        hws_c = np.ascontiguousarray(hw_pack[np.asarray(slot_tasks)])
        in_maps.append({
            "xT": xTp.reshape(KC, 128, BPC),
            "ctb": ct_c.reshape(KC, 128, BPC),
            "w0p": w0p, "w1p": w1p, "w2p": w2p, "bias": bias,
            "hws": hws_c, "msk": msk_c,
        })
    return in_maps, order, t_s, profile


def _assemble(results, order, t_s, head_b):
    head_b = np.asarray(head_b, np.float32)
    outs = []
    for c in range(NCORES):
        o = results[c]["outT"]                       # [NBLK, 128, HEAD_DIM]
        outs.append(o.reshape(BPC, HEAD_DIM))
    out_s = np.concatenate(outs, axis=0) + head_b[t_s]
    out = np.empty_like(out_s)
    out[order] = out_s
    return out.astype(np.float32)


def kernel(x, W0, b0, W1, b1, W2, b2, head_W, head_b):
    in_maps, order, t_s, prof = prepare(x, W0, b0, W1, b1, W2, b2, head_W, head_b)
    nc = _PROG_CACHE.get(prof)
    if nc is None:
        nc = _build(prof)
        _PROG_CACHE[prof] = nc
    res = bass_utils.run_bass_kernel_spmd(nc, in_maps, core_ids=list(range(NCORES)))
    return _assemble(res.results, order, t_s, head_b)


# revision 33
# speedup vs baseline: 1.4189x; 1.4189x over previous
"""Trainium2 Bass kernel for nn_MultiHeadNetwork (moe_routing).

Strategy
--------
Host side (numpy, inside kernel()):
  * task id per row = argmax of the trailing one-hot block of x (data, not
    activation dependent).  Rows are bin-packed by task into 32 blocks of 128
    rows (~2-3 distinct tasks each); blocks are then assigned to cores sorted
    by task count so a single shared per-position slot profile (e.g.
    (3, 3, 2, 2)) covers every core with minimal padding.
  * Trunk weights replicated across cores.  Per core the head only needs the
    few tasks its blocks span: per (block, slot) the spanned task's head
    weights are packed (the slot weights / masks are per-core data; the
    program structure is the shared profile, so SPMD-uniform).
  * Everything is converted to bfloat16 on host (halves DMA + H2D vs fp32;
    the PE runs bf16 at the same 1 col/cycle rate as fp32r, and bf16 enables
    fast weight loads).
  * The one-hot block of x contributes W0[2048+tid] + b0 per row, so layer 0
    runs over the 2048 dense features only (16 k-chunks instead of 17); the
    per-row contribution is host-gathered and added on the vector engine.

Device side (one SPMD Tile program on 8 cores):
  * Activations feature-major (hT: [feat partitions, batch free]) so each
    trunk layer is out = W_chunk.T @ hT with NO transposes anywhere.
  * Trunk layers run k-OUTER over 8-chunk groups (8 PSUM banks): weight DMAs
    are [128, 1024] bf16 (2 KiB / partition line), prefetched 14 deep; ReLU +
    bias fused on the scalar engine straight out of PSUM (layer 0: DVE add of
    the one-hot contribution + ReLU), output bf16.
  * A few dummy warm-up matmuls run during the first weight DMA so the PE
    HAM clock-gate is at full rate when the real work arrives.
  * Head: per 128-row block b and slot s: psum[row, h] accumulates
    h3[k][:, block cols].T @ head_W[slot][k-chunk] over the 16 k-chunks
    (the h3 block is the *stationary* operand).  copy_predicated with a host
    0/1 row mask merges each slot's rows into the block output.  All head
    weight DMAs are issued early so they complete during the trunk.
  * Head bias and the inverse permutation are applied on host.
"""

import numpy as np
import ml_dtypes
from contextlib import ExitStack

import concourse.bacc as bacc
import concourse.mybir as mybir
from concourse.tile import TileContext
from concourse import bass_utils

BATCH = 4096
FEAT = 2048
NUM_TASKS = 50
WIDTH = 2048
HEAD_DIM = 256
NCORES = 8
BPC = BATCH // NCORES          # 512 rows per core
BLK = 128                      # head row-block size
NBLK = BPC // BLK              # 4 blocks per core
KC = WIDTH // 128              # 16 contraction chunks per layer
WC = WIDTH // 128              # 16 w-chunks per layer
GW = 8                         # w-chunks per trunk group (8 PSUM banks)
NG = WC // GW                  # 2 groups per layer

F32 = mybir.dt.float32
BF16 = mybir.dt.bfloat16
U8 = mybir.dt.uint8
NPBF16 = ml_dtypes.bfloat16

_PROG_CACHE: dict = {}
_PACK_CACHE: dict = {}


def _fingerprint(*arrs):
    """Cheap identity+content fingerprint for caching packed weights."""
    parts = []
    for a in arrs:
        s = np.asarray(a).reshape(-1)
        step = max(1, s.size // 64)
        parts.append((id(a), s.size, float(s[::step][:64].astype(np.float64).sum())))
    return tuple(parts)


def _build(profile, repeat: int = 1, bench: bool = False):
    """Build + compile the SPMD Tile program.

    profile[p] = head slots for the core's p-th 128-row block (blocks are
    assigned to cores so that per-position task counts fit a single shared
    profile, e.g. (3, 3, 2, 2) -- smaller than a uniform max).  An int is
    accepted as a uniform profile.  repeat > 1 wraps the body in a hardware
    For_i loop (benchmarking only).  bench=True turns the big inputs into
    Internal (device-resident scratch) tensors so a benchmark run has no big
    H2D transfers.
    """
    if isinstance(profile, int):
        profile = (profile,) * NBLK
    sbase = [sum(profile[:p]) for p in range(NBLK)]
    nslot = sum(profile)
    nc = bacc.Bacc("TRN2", target_bir_lowering=False, debug=False)
    kind = "Internal" if bench else "ExternalInput"
    xT = nc.dram_tensor("xT", [KC, 128, BPC], BF16, kind=kind).ap()
    ctb = nc.dram_tensor("ctb", [KC, 128, BPC], BF16, kind=kind).ap()
    w0 = nc.dram_tensor("w0p", [NG, KC, 128, GW * 128], BF16, kind=kind).ap()
    w1 = nc.dram_tensor("w1p", [NG, KC, 128, GW * 128], BF16, kind=kind).ap()
    w2 = nc.dram_tensor("w2p", [NG, KC, 128, GW * 128], BF16, kind=kind).ap()
    bia = nc.dram_tensor("bias", [128, 3 * WC], F32, kind=kind).ap()
    hws = nc.dram_tensor("hws", [nslot, 128, KC * HEAD_DIM], BF16, kind=kind).ap()
    msk = nc.dram_tensor("msk", [128, nslot * HEAD_DIM], U8, kind=kind).ap()
    if bench:
        dummy = nc.dram_tensor("bmark_in", [128, 16], F32, kind="ExternalInput").ap()
        outk = "Internal"
    else:
        outk = "ExternalOutput"
    out = nc.dram_tensor("outT", [NBLK, 128, HEAD_DIM], F32, kind=outk).ap()
    if bench:
        outb = nc.dram_tensor("outb", [128, 16], F32, kind="ExternalOutput").ap()

    with TileContext(nc) as tc, ExitStack() as ctx:
        # xT (16 tiles) and h2 (16) share slots: h2 allocates only after
        # layer 0 fully finished reading xT.  h1/h3 share the other pool.
        actA = ctx.enter_context(tc.tile_pool(name="actA", bufs=KC))
        actB = ctx.enter_context(tc.tile_pool(name="actB", bufs=KC))
        wp = ctx.enter_context(tc.tile_pool(name="wp", bufs=14))
        cons = ctx.enter_context(tc.tile_pool(name="cons", bufs=1))
        ctp = ctx.enter_context(tc.tile_pool(name="ctp", bufs=10))
        hwp = ctx.enter_context(tc.tile_pool(name="hwp", bufs=nslot))
        op = ctx.enter_context(tc.tile_pool(name="op", bufs=4))
        psp = ctx.enter_context(tc.tile_pool(name="psp", bufs=8, space="PSUM"))

        if bench:
            # one-time (outside the loop) init of the Internal scratch
            # tensors: uninitialized HBM can hold NaN/denormal bit patterns
            # that would skew the timing vs real data
            with tc.tile_pool(name="initp", bufs=1) as initp:
                zt = initp.tile([128, GW * 128], BF16, tag="zt")
                nc.vector.memset(zt[:], 0.125)
                for g in range(NG):
                    for k in range(KC):
                        nc.sync.dma_start(w0[g, k], zt[:])
                        nc.sync.dma_start(w1[g, k], zt[:])
                        nc.sync.dma_start(w2[g, k], zt[:])
                for k in range(KC):
                    nc.sync.dma_start(xT[k], zt[:, :BPC])
                    nc.sync.dma_start(ctb[k], zt[:, :BPC])
                for sl in range(nslot):
                    for k in range(KC):
                        nc.sync.dma_start(
                            hws[sl][:, k * HEAD_DIM:(k + 1) * HEAD_DIM],
                            zt[:, :HEAD_DIM])
                ztf = initp.tile([128, 3 * WC], F32, tag="ztf")
                nc.vector.memset(ztf[:], 0.0)
                nc.sync.dma_start(bia, ztf[:])
                ztm = initp.tile([128, nslot * HEAD_DIM], U8, tag="ztm")
                nc.vector.memset(ztm[:], 1)
                nc.sync.dma_start(msk, ztm[:])

        if repeat > 1:
            ctx.enter_context(tc.For_i(0, repeat, 1))

        xt = [None] * KC

        # head-weight prefetch queue: dripped one DMA per few trunk
        # k-iterations (a single burst would starve the trunk weight DMAs)
        hw = [hwp.tile([128, KC * HEAD_DIM], BF16, tag="hwp", name=f"hw{sl}")
              for sl in range(nslot)]
        mt = cons.tile([128, nslot * HEAD_DIM], U8, tag="mt")
        bt = cons.tile([128, 3 * WC], F32, tag="bt")
        drip_q = [(mt, msk)] + [(hw[sl], hws[sl]) for sl in range(nslot)]

        # PE warm-up: dummy matmuls on a memset tile run during the initial
        # weight-DMA wait, so the HAM clock-gate un-throttles (needs ~3.4 us
        # of sustained PE activity) before the real matmuls begin.
        wu = wp.tile([128, GW * 128], BF16, tag="wp", name="warm")
        nc.vector.memset(wu[:], 0.0)
        wups = psp.tile([128, BPC], F32, tag="ps", name="warmps")
        for i in range(8):
            nc.tensor.matmul(wups[:], wu[:, :128], wu[:, :BPC],
                             start=(i == 0), stop=(i == 7))

        ctt = [None] * KC

        def trunk_layer(src, wdram, nk, li, pool, tag, load_x=False, drip=0):
            outs = [None] * WC
            it = 0
            for g in range(NG):
                pss = [
                    psp.tile([128, BPC], F32, tag="ps", name=f"psL{li}g{g}w{w}")
                    for w in range(GW)
                ]
                for k in range(nk):
                    wt = wp.tile([128, GW * 128], BF16, tag="wp", name=f"wtL{li}g{g}k{k}")
                    if li == 0 and g == 0 and k == 0:
                        # split the very first weight DMA so the first
                        # stationary [128,128] chunk lands as early as possible
                        nc.sync.dma_start(wt[:, :128], wdram[g, k][:, :128])
                        nc.sync.dma_start(wt[:, 128:], wdram[g, k][:, 128:])
                    else:
                        nc.sync.dma_start(wt[:], wdram[g, k])
                    if load_x and g == 0:
                        t = actA.tile([128, BPC], BF16, tag="actA", name=f"xt{k}")
                        nc.sync.dma_start(t[:], xT[k])
                        src[k] = t
                        if k == 0:
                            # bias: small DMA, must be traced before the
                            # first ReLU that reads it
                            nc.sync.dma_start(bt[:], bia)
                    if load_x and k % 2 == 1:
                        # drip the one-hot contribution tiles for this group:
                        # chunk g*GW + (k//2) is needed by this group's ReLU
                        ci = g * GW + k // 2
                        ctt[ci] = ctp.tile([128, BPC], BF16, tag="ctp",
                                           name=f"ct{ci}")
                        nc.sync.dma_start(ctt[ci][:], ctb[ci])
                    if drip and drip_q and it % drip == drip - 1:
                        tile, src_ap = drip_q.pop(0)
                        nc.sync.dma_start(tile[:], src_ap)
                    it += 1
                    for w in range(GW):
                        nc.tensor.matmul(
                            pss[w][:],
                            wt[:, w * 128:(w + 1) * 128],
                            src[k][:],
                            start=(k == 0),
                            stop=(k == nk - 1),
                        )
                for w in range(GW):
                    wc_i = g * GW + w
                    h = pool.tile([128, BPC], BF16, tag=tag, name=f"h{li}_{wc_i}")
                    if li == 0:
                        # layer 0: the one-hot block of x contributes
                        # W0[2048+tid] + b0 per row (host-gathered): add on
                        # DVE straight out of PSUM, then ReLU in place
                        nc.vector.tensor_tensor(
                            h[:], pss[w][:], ctt[wc_i][:], mybir.AluOpType.add)
                        nc.vector.tensor_relu(h[:], h[:])
                    else:
                        nc.scalar.activation(
                            h[:], pss[w][:], mybir.ActivationFunctionType.Relu,
                            bias=bt[:, li * WC + wc_i: li * WC + wc_i + 1],
                        )
                    outs[wc_i] = h
            return outs

        h1 = trunk_layer(xt, w0, KC, 0, actB, "actB", load_x=True)
        # lighter drip during L1 (its wt DMAs + drips are the heaviest HBM
        # stretch under 8-core contention), denser during L2
        h2 = trunk_layer(h1, w1, KC, 1, actA, "actA", drip=3)
        h3 = trunk_layer(h2, w2, KC, 2, actB, "actB", drip=2)
        for tile, src_ap in drip_q:
            nc.sync.dma_start(tile[:], src_ap)
        drip_q.clear()

        for b in range(NBLK):
            ob = op.tile([128, HEAD_DIM], F32, tag="op", name=f"ob{b}")
            for s in range(profile[b]):
                sl = sbase[b] + s
                ps = psp.tile([128, HEAD_DIM], F32, tag="ps", name=f"psH{sl}")
                for k in range(KC):
                    nc.tensor.matmul(
                        ps[:],
                        h3[k][:, b * BLK:(b + 1) * BLK],
                        hw[sl][:, k * HEAD_DIM:(k + 1) * HEAD_DIM],
                        start=(k == 0),
                        stop=(k == KC - 1),
                    )
                if s == 0:
                    nc.vector.tensor_copy(ob[:], ps[:])
                else:
                    nc.vector.copy_predicated(
                        ob[:], mt[:, sl * HEAD_DIM:(sl + 1) * HEAD_DIM], ps[:]
                    )
            nc.sync.dma_start(out[b], ob[:])

        if bench:
            dt = cons.tile([128, 16], F32, tag="dt")
            nc.sync.dma_start(dt[:], dummy)
            nc.vector.tensor_copy(dt[:], ob[:, :16])
            nc.sync.dma_start(outb, dt[:])

    nc.compile()
    return nc


def _pack_w(W, nk):
    # [NG, nk, 128, GW*128]; [g, k, kp, w*128+m] = W[k*128+kp, (g*GW+w)*128+m]
    return np.ascontiguousarray(
        W.reshape(nk, 128, NG, GW * 128).transpose(2, 0, 1, 3)
    )


def _pack_trunk(W0, W1, W2, b1, b2):
    w0p = _pack_w(W0[:FEAT].astype(NPBF16), KC)
    w1p = _pack_w(W1.astype(NPBF16), KC)
    w2p = _pack_w(W2.astype(NPBF16), KC)
    bias = np.zeros((128, 3 * WC), np.float32)
    for li, b in ((1, b1), (2, b2)):
        bias[:, li * WC:(li + 1) * WC] = b.reshape(WC, 128).T
    return w0p, w1p, w2p, bias


def _pack_rows(tid):
    """Bin-pack rows by task into 128-row blocks, then assign blocks to
    cores so that per-position task counts are balanced: the shared slot
    profile becomes e.g. (3, 3, 2, 2) instead of a uniform max of 3.

    Returns (order, core_blocks, profile): order[i] = original row index at
    packed position i (rows laid out core-major, position-major);
    core_blocks[c][p] = task list of core c's p-th block; profile[p] = slot
    count for position p.
    """
    nblk_total = BATCH // BLK
    counts = np.bincount(tid, minlength=NUM_TASKS)
    rem = {t: int(c) for t, c in enumerate(counts) if c > 0}
    # queues of original row indices per task
    row_q = {t: list(np.nonzero(tid == t)[0]) for t in rem}
    blocks = []  # (row_indices, task_list)
    carry = None  # (task, count) remnant that must start the next block
    for b in range(nblk_total):
        cap = BLK
        rows_here = []
        tasks_here = []
        while cap > 0:
            if carry is not None:
                t, c = carry
                carry = None
            elif rem:
                # largest task that fits entirely, else split the largest
                fit = [(c, t) for t, c in rem.items() if c <= cap]
                if fit:
                    c, t = max(fit)
                else:
                    c, t = max((c, t) for t, c in rem.items())
                del rem[t]
            else:
                break
            take = min(c, cap)
            q = row_q[t]
            rows_here.extend(q[:take])
            row_q[t] = q[take:]
            if t not in tasks_here:
                tasks_here.append(t)
            cap -= take
            if c > take:
                carry = (t, c - take)
        blocks.append((rows_here, tasks_here))
    assert carry is None and not rem
    # sort blocks by task count desc; position p of core c gets global rank
    # 8p + c, so within each core counts are non-increasing and profile[p]
    # (the per-position max) is as small as the distribution allows
    ranked = sorted(range(nblk_total), key=lambda b: -len(blocks[b][1]))
    core_blocks = [[None] * NBLK for _ in range(NCORES)]
    order = []
    profile = []
    for p in range(NBLK):
        profile.append(len(blocks[ranked[8 * p]][1]))
    for c in range(NCORES):
        for p in range(NBLK):
            rows, tasks = blocks[ranked[8 * p + c]]
            core_blocks[c][p] = tasks
            order.extend(rows)
    return np.asarray(order), core_blocks, tuple(profile)


def prepare(x, W0, b0, W1, b1, W2, b2, head_W, head_b):
    """Host-side sharding. Returns (in_maps, order, sorted_task_ids, profile)."""
    x = np.asarray(x, np.float32)
    tid = np.argmax(x[:, -NUM_TASKS:], axis=1)
    order, core_blocks, profile = _pack_rows(tid)
    x_s = x[order]
    t_s = tid[order]
    sbase = [sum(profile[:p]) for p in range(NBLK)]

    fp = _fingerprint(W0, W1, W2, b0, b1, b2, head_W)
    cached = _PACK_CACHE.get("w")
    if cached is not None and cached[0] == fp:
        w0p, w1p, w2p, bias, W0oh, hw_pack = cached[1]
    else:
        W0 = np.asarray(W0, np.float32)
        w0p, w1p, w2p, bias = _pack_trunk(
            W0, np.asarray(W1, np.float32), np.asarray(W2, np.float32),
            np.asarray(b1, np.float32), np.asarray(b2, np.float32))
        # one-hot contribution rows: relu(x @ W0 + b0) = relu(feats @
        # W0[:2048] + W0[2048 + tid] + b0) -- last two terms host-gathered
        W0oh = W0[FEAT:FEAT + NUM_TASKS] + np.asarray(b0, np.float32)[None, :]
        head_W = np.asarray(head_W, np.float32).astype(NPBF16)
        # hw_pack[t, kp, kc*256 + j] = head_W[t, kc*128 + kp, j]
        hw_pack = np.ascontiguousarray(
            head_W.reshape(NUM_TASKS, KC, 128, HEAD_DIM)
            .transpose(0, 2, 1, 3)
            .reshape(NUM_TASKS, 128, KC * HEAD_DIM)
        )
        _PACK_CACHE["w"] = (fp, (w0p, w1p, w2p, bias, W0oh, hw_pack))

    nslot = sum(profile)
    in_maps = []
    for c in range(NCORES):
        xs = x_s[c * BPC:(c + 1) * BPC]
        xTp = np.ascontiguousarray(xs[:, :FEAT].T).astype(NPBF16)
        ts_c = t_s[c * BPC:(c + 1) * BPC]
        ct_c = np.ascontiguousarray(W0oh[ts_c].T).astype(NPBF16)
        slot_tasks = []
        msk_c = np.zeros((128, nslot * HEAD_DIM), np.uint8)
        for b in range(NBLK):
            tl = core_blocks[c][b]
            tl_p = tl + [tl[-1]] * (profile[b] - len(tl))
            ch = t_s[c * BPC + b * BLK: c * BPC + (b + 1) * BLK]
            for s, t in enumerate(tl_p):
                sl = sbase[b] + s
                slot_tasks.append(t)
                if 0 < s < len(tl):
                    msk_c[:, sl * HEAD_DIM:(sl + 1) * HEAD_DIM] = \
                        (ch == t)[:, None].astype(np.uint8)
        hws_c = np.ascontiguousarray(hw_pack[np.asarray(slot_tasks)])
        in_maps.append({
            "xT": xTp.reshape(KC, 128, BPC),
            "ctb": ct_c.reshape(KC, 128, BPC),
            "w0p": w0p, "w1p": w1p, "w2p": w2p, "bias": bias,
            "hws": hws_c, "msk": msk_c,
        })
    return in_maps, order, t_s, profile


def _assemble(results, order, t_s, head_b):
    head_b = np.asarray(head_b, np.float32)
    outs = []
    for c in range(NCORES):
        o = results[c]["outT"]                       # [NBLK, 128, HEAD_DIM]
        outs.append(o.reshape(BPC, HEAD_DIM))
    out_s = np.concatenate(outs, axis=0) + head_b[t_s]
    out = np.empty_like(out_s)
    out[order] = out_s
    return out.astype(np.float32)


def kernel(x, W0, b0, W1, b1, W2, b2, head_W, head_b):
    in_maps, order, t_s, prof = prepare(x, W0, b0, W1, b1, W2, b2, head_W, head_b)
    nc = _PROG_CACHE.get(prof)
    if nc is None:
        nc = _build(prof)
        _PROG_CACHE[prof] = nc
    res = bass_utils.run_bass_kernel_spmd(nc, in_maps, core_ids=list(range(NCORES)))
    return _assemble(res.results, order, t_s, head_b)
